# revision 1
# baseline (speedup 1.0000x reference)
import numpy as np

# GATv2 backbone: N=50000 nodes, E=400000 edges (+N self loops), 3 layers.
# Layers 0/1: H=4 heads, C=16 per-head channels, concat -> 64, ELU after.
# Layer 2: H=4 heads, C=64, mean over heads -> 64.
N = 50000
H = 4
NEG_SLOPE = 0.2
EPS = 1e-16


def _leaky_relu(v):
    return np.where(v >= 0.0, v, v * NEG_SLOPE).astype(np.float32)


def _elu(v):
    return np.where(v >= 0.0, v, np.expm1(np.minimum(v, 0.0))).astype(np.float32)


def _gatv2_layer(h, s_src, starts, Wl, bl, Wr, br, att, bias, concat):
    heads, c = att.shape
    xl = (h @ Wl + bl).reshape(N, heads, c)
    xr = (h @ Wr + br).reshape(N, heads, c)
    # edges are pre-sorted by destination; starts[i] is the first edge of node i
    # (every node has a self loop, so all N segments are non-empty)
    e = _leaky_relu(xl[s_src] + np.repeat(xr, np.diff(np.append(starts, s_src.shape[0])), axis=0))
    alpha = np.einsum('ehc,hc->eh', e, att).astype(np.float32)
    amax = np.maximum.reduceat(alpha, starts, axis=0)
    ealpha = np.exp(alpha - np.repeat(amax, np.diff(np.append(starts, s_src.shape[0])), axis=0))
    denom = np.add.reduceat(ealpha, starts, axis=0)
    alphan = ealpha / (np.repeat(denom, np.diff(np.append(starts, s_src.shape[0])), axis=0) + EPS)
    msg = (xl[s_src] * alphan[:, :, None]).reshape(-1, heads * c)
    out = np.add.reduceat(msg, starts, axis=0).reshape(N, heads, c)
    if concat:
        return (out.reshape(N, heads * c) + bias).astype(np.float32)
    return (out.mean(axis=1) + bias).astype(np.float32)


def kernel(x, edge_index, Wl0, bl0, Wr0, br0, att0, bias0,
           Wl1, bl1, Wr1, br1, att1, bias1,
           Wl2, bl2, Wr2, br2, att2, bias2):
    x = np.asarray(x, np.float32)
    ei = np.asarray(edge_index)
    loops = np.arange(N, dtype=ei.dtype)
    src = np.concatenate([ei[0], loops])
    dst = np.concatenate([ei[1], loops])
    order = np.argsort(dst, kind='stable')
    s_src = src[order]
    s_dst = dst[order]
    # first-edge offsets per destination node; all nodes present (self loops)
    starts = np.searchsorted(s_dst, np.arange(N, dtype=s_dst.dtype))

    h = _gatv2_layer(x, s_src, starts,
                     np.asarray(Wl0, np.float32), np.asarray(bl0, np.float32),
                     np.asarray(Wr0, np.float32), np.asarray(br0, np.float32),
                     np.asarray(att0, np.float32), np.asarray(bias0, np.float32), True)
    h = _elu(h)
    h = _gatv2_layer(h, s_src, starts,
                     np.asarray(Wl1, np.float32), np.asarray(bl1, np.float32),
                     np.asarray(Wr1, np.float32), np.asarray(br1, np.float32),
                     np.asarray(att1, np.float32), np.asarray(bias1, np.float32), True)
    h = _elu(h)
    h = _gatv2_layer(h, s_src, starts,
                     np.asarray(Wl2, np.float32), np.asarray(bl2, np.float32),
                     np.asarray(Wr2, np.float32), np.asarray(br2, np.float32),
                     np.asarray(att2, np.float32), np.asarray(bias2, np.float32), False)
    return h



# revision 8
# speedup vs baseline: 4.2837x; 4.2837x over previous
"""GATv2 backbone (3 layers, H=4) on 8 Trainium2 NeuronCores via Bass/Tile.

Sharding: destination-node-range sharding (edges grouped by dst block).
Each core owns a contiguous range of destination nodes; edges are sorted by
dst on host and grouped into 128-node destination blocks, padded to a fixed
number of 128-edge tiles per block (T_b) so all cores run one identical
program (SPMD). Per-edge messages/exp-scores are aggregated per block with a
selection-matrix matmul into PSUM. Node features for the next layer are
exchanged with an in-kernel AllGather (transposed, bf16).
"""

import functools
import math

import numpy as np
import ml_dtypes

N_NODES = 50000
N_EDGES = 400000
DIN = 64
H = 4
NC = 8
P = 128
NEG_SLOPE = 0.2

BF16 = ml_dtypes.bfloat16

# layer dims: (D = heads*channels of xl/xr, C per-head, concat?)
LAYERS = [(64, 16, True), (64, 16, True), (256, 64, False)]


def _plan(n):
    bc = math.ceil(n / (NC * P))
    npad = NC * bc * P
    rc = bc * P
    return bc, npad, rc


def _preprocess(n, src, dst):
    """Sort edges by dst, group into 128-node blocks, pad each block to a
    multiple of 128 edge slots. Returns T_b and per-core index arrays."""
    bc, npad, rc = _plan(n)
    nblk = NC * bc
    order = np.argsort(dst, kind="stable")
    s_src = src[order].astype(np.int64)
    s_dst = dst[order].astype(np.int64)
    blk = s_dst >> 7  # dst // 128
    cnt = np.bincount(blk, minlength=nblk).astype(np.int64)
    t_b = max(1, int(math.ceil(cnt.max() / P)))
    sl = t_b * P
    starts = np.zeros(nblk, np.int64)
    starts[1:] = np.cumsum(cnt)[:-1]
    # slot position of each edge inside its block
    pos = np.arange(len(s_dst), dtype=np.int64) - starts[blk]
    srcg = np.zeros((nblk, sl), np.int32)
    dstl = np.zeros((nblk, sl), np.int32)
    ldst = np.full((nblk, sl), 128.0, np.float32)
    srcg[blk, pos] = s_src.astype(np.int32)
    core_of = blk // bc
    dstl[blk, pos] = (s_dst - core_of * rc).astype(np.int32)
    ldst[blk, pos] = (s_dst & 127).astype(np.float32)
    # [nblk, sl] -> [NC, bc, P, t_b]: slot s = t*P + p
    def shape(a):
        return np.ascontiguousarray(
            a.reshape(NC, bc, t_b, P).transpose(0, 1, 3, 2)
        )
    return t_b, shape(srcg), shape(dstl), shape(ldst).astype(BF16)


@functools.lru_cache(maxsize=4)
def _program(n, t_b, nlayers=3, dbg=False):
    import concourse.bacc as bacc
    import concourse.bass as bass
    import concourse.mybir as mybir
    import concourse.tile as tile
    from concourse.masks import make_identity

    bc, npad, rc = _plan(n)
    nt = npad // P  # node tiles, full range
    f32 = mybir.dt.float32
    bf16 = mybir.dt.bfloat16
    i32 = mybir.dt.int32
    AX = mybir.AxisListType.X
    OP = mybir.AluOpType
    AF = mybir.ActivationFunctionType

    nc = bacc.Bacc("TRN2", target_bir_lowering=False, debug=False, num_devices=NC)

    # ---- external I/O ----
    xT_in = nc.dram_tensor("xT", [DIN + 1, npad], bf16, kind="ExternalInput")
    xTown_in = nc.dram_tensor("xTown", [DIN + 1, rc], bf16, kind="ExternalInput")
    srcg_in = nc.dram_tensor("srcg", [bc, P, t_b], i32, kind="ExternalInput")
    dstl_in = nc.dram_tensor("dstl", [bc, P, t_b], i32, kind="ExternalInput")
    ldst_in = nc.dram_tensor("ldst", [bc, P, t_b], bf16, kind="ExternalInput")
    riota_in = nc.dram_tensor("riota", [P, P], bf16, kind="ExternalInput")
    w_in, att_in, ob_in = {}, {}, {}
    for li, (D, C, _) in enumerate(LAYERS):
        w_in[li] = (
            nc.dram_tensor(f"wl{li}", [DIN + 1, D], bf16, kind="ExternalInput"),
            nc.dram_tensor(f"wr{li}", [DIN + 1, D], bf16, kind="ExternalInput"),
        )
        att_in[li] = nc.dram_tensor(f"attb{li}", [P, D], f32, kind="ExternalInput")
        ob_in[li] = nc.dram_tensor(f"ob{li}", [P, DIN], f32, kind="ExternalInput")
    y_out = nc.dram_tensor("y", [rc, DIN], f32, kind="ExternalOutput")

    with tile.TileContext(nc) as tc:
        with (
            tc.tile_pool(name="const", bufs=1) as cpool,
            tc.tile_pool(name="big", bufs=1) as bigpool,
            tc.tile_pool(name="own", bufs=2) as ownpool,
            tc.tile_pool(name="work", bufs=2) as wpool,
            tc.tile_pool(name="fin", bufs=3) as fpool,
            tc.tile_pool(name="psx", bufs=2, space="PSUM") as psx,
            tc.tile_pool(name="psb", bufs=2, space="PSUM") as psb,
            tc.tile_pool(name="pst", bufs=2, space="PSUM") as pst,
            tc.tile_pool(name="dram", bufs=1, space="DRAM") as dpool,
        ):
            # ---- constants ----
            ident = cpool.tile([P, P], bf16, name="ident")
            make_identity(nc, ident[:])
            riota = cpool.tile([P, P], bf16, name="riota_t")
            nc.sync.dma_start(riota[:], riota_in[:])
            wsb, attsb, obsb = {}, {}, {}
            for li, (D, C, _) in enumerate(LAYERS):
                wl = cpool.tile([DIN + 1, D], bf16, name=f"wl{li}_t")
                wr = cpool.tile([DIN + 1, D], bf16, name=f"wr{li}_t")
                nc.sync.dma_start(wl[:], w_in[li][0][:])
                nc.sync.dma_start(wr[:], w_in[li][1][:])
                wsb[li] = (wl, wr)
                ab = cpool.tile([P, D], f32, name=f"attb{li}_t")
                nc.sync.dma_start(ab[:], att_in[li][:])
                attsb[li] = ab
                obt = cpool.tile([P, DIN], f32, name=f"ob{li}_t")
                nc.sync.dma_start(obt[:], ob_in[li][:])
                obsb[li] = obt

            # ---- per-layer DRAM feature tensors (layer1 reuses layer0's) ----
            xl_d, xr_d = {}, {}
            xl_d[0] = dpool.tile([npad, 64], bf16, name="xl01_d")
            xr_d[0] = dpool.tile([rc, 64], bf16, name="xr01_d")
            xl_d[1], xr_d[1] = xl_d[0], xr_d[0]
            xl_d[2] = dpool.tile([npad, 256], bf16, name="xl2_d")
            xr_d[2] = dpool.tile([rc, 256], bf16, name="xr2_d")

            # full-range transposed features (lhsT source), ones row at 64
            xTf = bigpool.tile([DIN + 1, npad], bf16, name="xTf", tag="xTf")
            nc.sync.dma_start(xTf[:], xT_in[:])
            own = ownpool.tile([DIN + 1, rc], bf16, name="own0", tag="own")
            nc.sync.dma_start(own[:], xTown_in[:])

            for li, (D, C, concat) in enumerate(LAYERS[:nlayers]):
                wl, wr = wsb[li]
                attb, obias = attsb[li], obsb[li]
                xb = max(1, 512 // D)  # node tiles per PSUM flush

                # ---- xl pass (full range) + xr pass (own range) ----
                for which, lhs_src, n_tiles, dst_d in (
                    ("xl", xTf, nt, xl_d[li]),
                    ("xr", own, bc, xr_d[li]),
                ):
                    w_t = wl if which == "xl" else wr
                    for c0 in range(0, n_tiles, xb):
                        nb = min(xb, n_tiles - c0)
                        px = psx.tile([P, xb * D], f32, name="px", tag="px")
                        for j in range(nb):
                            ntile = c0 + j
                            nc.tensor.matmul(
                                px[:, j * D:(j + 1) * D],
                                lhsT=lhs_src[:, ntile * P:(ntile + 1) * P],
                                rhs=w_t[:],
                                start=True,
                                stop=True,
                            )
                        xlr = wpool.tile([P, xb, D], bf16, name="xlr", tag="xlr")
                        nc.any.tensor_copy(
                            xlr[:, :nb, :],
                            px[:, : nb * D].rearrange("p (t d) -> p t d", d=D),
                        )
                        nc.sync.dma_start(
                            dst_d[c0 * P:(c0 + nb) * P, :].rearrange(
                                "(t p) d -> p t d", p=P
                            ),
                            xlr[:, :nb, :],
                        )

                # ---- edge phase per destination block ----
                own_next = None
                if li < 2:
                    own_next = ownpool.tile([DIN + 1, rc], bf16,
                                            name=f"own{li + 1}", tag="own")
                    nc.vector.memset(own_next[DIN:DIN + 1, :], 1.0)

                for b in range(bc):
                    srcg = wpool.tile([P, t_b], i32, name="srcg_t", tag="srcg")
                    dstl = wpool.tile([P, t_b], i32, name="dstl_t", tag="dstl")
                    ldst = wpool.tile([P, t_b], bf16, name="ldst_t", tag="ldst")
                    nc.sync.dma_start(srcg[:], srcg_in[b])
                    nc.sync.dma_start(dstl[:], dstl_in[b])
                    nc.sync.dma_start(ldst[:], ldst_in[b])

                    mblk = wpool.tile([P, t_b, P], bf16, name="mblk", tag="mblk")
                    nc.vector.tensor_tensor(
                        out=mblk[:],
                        in0=ldst[:].unsqueeze(2).to_broadcast([P, t_b, P]),
                        in1=riota[:].unsqueeze(1).to_broadcast([P, t_b, P]),
                        op=OP.is_equal,
                    )

                    xlg = wpool.tile([P, t_b, D], bf16, name="xlg", tag="xlg")
                    xrg = wpool.tile([P, t_b, D], bf16, name="xrg", tag="xrg")
                    for t in range(t_b):
                        nc.gpsimd.indirect_dma_start(
                            out=xlg[:, t, :],
                            out_offset=None,
                            in_=xl_d[li][:],
                            in_offset=bass.IndirectOffsetOnAxis(
                                ap=srcg[:, t:t + 1], axis=0
                            ),
                        )
                        nc.gpsimd.indirect_dma_start(
                            out=xrg[:, t, :],
                            out_offset=None,
                            in_=xr_d[li][:],
                            in_offset=bass.IndirectOffsetOnAxis(
                                ap=dstl[:, t:t + 1], axis=0
                            ),
                        )

                    vt = wpool.tile([P, t_b, D], f32, name="vt", tag="vt")
                    nc.vector.tensor_tensor(out=vt[:], in0=xlg[:], in1=xrg[:],
                                            op=OP.add)
                    nc.scalar.activation(vt[:], vt[:], AF.Prelu,
                                         alpha=NEG_SLOPE)
                    nc.vector.tensor_tensor(
                        out=vt[:],
                        in0=vt[:],
                        in1=attb[:].unsqueeze(1).to_broadcast([P, t_b, D]),
                        op=OP.mult,
                    )
                    alph = wpool.tile([P, t_b, H], f32, name="alph", tag="alph")
                    nc.vector.reduce_sum(
                        out=alph[:],
                        in_=vt[:].rearrange("p t (h c) -> p t h c", c=C),
                        axis=AX,
                    )
                    payload = wpool.tile([P, t_b, D + H], bf16, name="payload",
                                         tag="payload")
                    nc.scalar.activation(payload[:, :, D:D + H], alph[:], AF.Exp)
                    nc.vector.tensor_tensor(
                        out=payload[:, :, 0:D].rearrange(
                            "p t (h c) -> p t h c", c=C),
                        in0=xlg[:].rearrange("p t (h c) -> p t h c", c=C),
                        in1=payload[:, :, D:D + H].unsqueeze(3).to_broadcast(
                            [P, t_b, H, C]),
                        op=OP.mult,
                    )

                    pblk = psb.tile([P, D + H], f32, name="pblk", tag="pblk")
                    for t in range(t_b):
                        nc.tensor.matmul(
                            pblk[:],
                            lhsT=mblk[:, t, :],
                            rhs=payload[:, t, :],
                            start=(t == 0),
                            stop=(t == t_b - 1),
                        )

                    rden = fpool.tile([P, H], f32, name="rden", tag="rden")
                    nc.vector.reciprocal(rden[:], pblk[:, D:D + H])
                    z = fpool.tile([P, D], f32, name="z", tag="z")
                    nc.vector.tensor_tensor(
                        out=z[:].rearrange("p (h c) -> p h c", c=C),
                        in0=pblk[:, 0:D].rearrange("p (h c) -> p h c", c=C),
                        in1=rden[:].unsqueeze(2).to_broadcast([P, H, C]),
                        op=OP.mult,
                    )
                    if concat:
                        zb = fpool.tile([P, D], f32, name="zb", tag="zb")
                        nc.vector.tensor_tensor(out=zb[:], in0=z[:],
                                                in1=obias[:], op=OP.add)
                        at = fpool.tile([P, D], f32, name="at", tag="at")
                        nc.scalar.activation(at[:], zb[:], AF.Relu)
                        mn = fpool.tile([P, D], f32, name="mn", tag="mn")
                        nc.vector.tensor_scalar_min(mn[:], zb[:], 0.0)
                        et = fpool.tile([P, D], f32, name="et", tag="et")
                        nc.scalar.activation(et[:], mn[:], AF.Exp)
                        st = fpool.tile([P, D], f32, name="st", tag="st")
                        nc.vector.tensor_tensor(out=st[:], in0=at[:], in1=et[:],
                                                op=OP.add)
                        hb = fpool.tile([P, D], bf16, name="hb", tag="hb")
                        nc.vector.tensor_scalar_add(hb[:], st[:], -1.0)
                        pt = pst.tile([DIN, P], bf16, name="pt", tag="pt")
                        nc.tensor.transpose(out=pt[:], in_=hb[:],
                                            identity=ident[:])
                        nc.any.tensor_copy(
                            own_next[0:DIN, b * P:(b + 1) * P], pt[:])
                    else:
                        red = fpool.tile([P, DIN], f32, name="red", tag="red")
                        nc.vector.reduce_sum(
                            out=red[:],
                            in_=z[:].rearrange("p (h c) -> p c h", c=C),
                            axis=AX,
                        )
                        outf = fpool.tile([P, DIN], f32, name="outf", tag="outf")
                        nc.scalar.activation(outf[:], red[:], AF.Copy,
                                             scale=1.0 / H)
                        nc.vector.tensor_tensor(out=outf[:], in0=outf[:],
                                                in1=obias[:], op=OP.add)
                        nc.sync.dma_start(y_out[b * P:(b + 1) * P, :], outf[:])

                if dbg:
                    dxl = nc.dram_tensor(f"dxl{li}", [npad, D], bf16,
                                         kind="ExternalOutput")
                    dxr = nc.dram_tensor(f"dxr{li}", [rc, D], bf16,
                                         kind="ExternalOutput")
                    nc.sync.dma_start(dxl[:], xl_d[li][:])
                    nc.sync.dma_start(dxr[:], xr_d[li][:])
                    if li < 2:
                        dho = nc.dram_tensor(f"dho{li}", [DIN, rc], bf16,
                                             kind="ExternalOutput")
                        nc.sync.dma_start(dho[:], own_next[0:DIN, :])

                # ---- exchange h for next layer ----
                if li < 2:
                    htd = dpool.tile([DIN, rc], bf16, name=f"htd{li}")
                    nc.sync.dma_start(htd[:], own_next[0:DIN, :])
                    hta = dpool.tile([NC, DIN, rc], bf16, name=f"hta{li}",
                                     addr_space="Shared")
                    nc.gpsimd.collective_compute(
                        "AllGather",
                        OP.bypass,
                        replica_groups=[list(range(NC))],
                        ins=[htd.opt()],
                        outs=[hta.opt()],
                    )
                    xTf = bigpool.tile([DIN + 1, npad], bf16,
                                       name=f"xTf{li + 1}", tag="xTf")
                    nc.sync.dma_start(
                        xTf[0:DIN, :].rearrange("p (r c) -> p r c", r=NC),
                        hta[:].transpose([1, 0, 2]),
                    )
                    nc.vector.memset(xTf[DIN:DIN + 1, :], 1.0)
                    own = own_next

    nc.finalize()
    return nc


def _prep_inputs(n, x, srcg, dstl, ldst, params):
    """Build the 8 per-core input maps."""
    bc, npad, rc = _plan(n)
    xT = np.zeros((DIN + 1, npad), BF16)
    xT[:DIN, :n] = np.asarray(x, np.float32).T.astype(BF16)
    xT[DIN, :] = BF16(1.0)
    riota = np.broadcast_to(
        np.arange(P, dtype=np.float32), (P, P)).astype(BF16).copy()
    common = {"xT": xT, "riota": riota}
    for li, (D, C, _) in enumerate(LAYERS):
        Wl, bl, Wr, br, att, bias = params[li]
        common[f"wl{li}"] = np.concatenate(
            [np.asarray(Wl, np.float32),
             np.asarray(bl, np.float32)[None, :]], 0).astype(BF16)
        common[f"wr{li}"] = np.concatenate(
            [np.asarray(Wr, np.float32),
             np.asarray(br, np.float32)[None, :]], 0).astype(BF16)
        common[f"attb{li}"] = np.broadcast_to(
            np.asarray(att, np.float32).reshape(1, D), (P, D)).copy()
        ob = np.zeros((P, DIN), np.float32)
        ob[:] = np.asarray(bias, np.float32)[None, :]
        common[f"ob{li}"] = ob
    in_maps = []
    for c in range(NC):
        m = dict(common)
        m["xTown"] = np.ascontiguousarray(xT[:, c * rc:(c + 1) * rc])
        m["srcg"] = srcg[c]
        m["dstl"] = dstl[c]
        m["ldst"] = ldst[c]
        in_maps.append(m)
    return in_maps


def _run(n, in_maps, t_b, nlayers=3, dbg=False, raw=False):
    from concourse.bass_utils import run_bass_kernel_spmd

    nc = _program(n, t_b, nlayers, dbg)
    res = run_bass_kernel_spmd(nc, in_maps, core_ids=list(range(NC)))
    if raw:
        return res
    return np.concatenate([res.results[c]["y"] for c in range(NC)], 0)


def kernel(x, edge_index, Wl0, bl0, Wr0, br0, att0, bias0,
           Wl1, bl1, Wr1, br1, att1, bias1,
           Wl2, bl2, Wr2, br2, att2, bias2):
    n = x.shape[0]
    ei = np.asarray(edge_index).astype(np.int64)
    loops = np.arange(n, dtype=np.int64)
    src = np.concatenate([ei[0], loops])
    dst = np.concatenate([ei[1], loops])
    t_b, srcg, dstl, ldst = _preprocess(n, src, dst)
    params = [
        (Wl0, bl0, Wr0, br0, att0, bias0),
        (Wl1, bl1, Wr1, br1, att1, bias1),
        (Wl2, bl2, Wr2, br2, att2, bias2),
    ]
    in_maps = _prep_inputs(n, x, srcg, dstl, ldst, params)
    out = _run(n, in_maps, t_b)
    return np.ascontiguousarray(out[:n]).astype(np.float32)
